# revision 1
# baseline (speedup 1.0000x reference)
"""Chi2 loss over ragged windows — Trainium2 Bass kernel.

Math (per sample b of B=4096, rows of length L=4096):
    len  = e_in - s_in            (in [1024, 3072])
    chi2 = sum_{j<len} ivar[b, s_in+j] * (flu[b, s_in+j] - out[b, s_out+j])^2
    result = mean_b(chi2 / len)

Strategy: pure data-parallel over the batch, 512 samples per core on 8
cores. The three arrays are concatenated into one DRAM tensor per core;
each 128-sample tile is fetched with a single indirect DMA (3 window
chunks per sample row, offsets precomputed on host), aligned so position
j holds flu[s_in+j] / ivar[s_in+j] / out[s_out+j]. On-chip: d = x - y,
d2 = d^2 (ACT), a j < len mask zeroes the ragged tail, prod = d2 * w *
mask, and a per-partition reduction produces one partial sum per sample.
Host divides by len and takes the global mean.

Perf shape (from cost-model timeline iteration):
  - samples sorted by len inside each core; tile t's gather is only as
    wide as its longest window (rounded to 128, shared across cores for
    the single SPMD program) — ~25% less HBM traffic.
  - each tile is split into a maskless "base" piece (columns below the
    tile's min len, always valid) and a masked "tail" piece.
  - masked tail pieces transfer first (high compute/byte), maskless
    bases last, so the DVE never accumulates a backlog and the exposed
    tail after the final transfer is one short chain.
  - the final base piece is split in two and the last two pieces compute
    entirely on the DVE (square/reduce instead of ACT) to avoid
    cross-engine semaphore hops in the drain.
  - SWDGE descriptor ring enlarged (32KB/partition) so descriptor
    generation runs arbitrarily far ahead of the transfers.
"""

import numpy as np

import bass_rust
import concourse.bass as bass
import concourse.tile as tile
from concourse import mybir
from concourse.bass_utils import run_bass_kernel_spmd
from concourse.tile_rust import add_dep_helper

B, L = 4096, 4096
N_CORES = 8
BPC = B // N_CORES          # samples per core
P = 128                     # SBUF partitions
TILES = BPC // P            # 128-sample tiles per core
MAX_W = 3072                # max window length
ROWS = 3 * (BPC + 1)        # concat of flu/ivr/oup shards, each padded 1 row

f32 = mybir.dt.float32
i32 = mybir.dt.int32


def legalize_waits(nc):
    """This compiler build only accepts one sync wait per instruction; hoist
    extra waits into standalone single-wait EventSemaphore instructions."""
    n = 0
    for func in nc.m.functions:
        for blk in func.blocks:
            insts = blk.instructions
            out = []
            for inst in insts:
                si = inst.sync_info
                if si is not None and si.on_wait and len(si.on_wait) > 1:
                    waits = list(si.on_wait)
                    for w in waits[:-1]:
                        n += 1
                        out.append(
                            bass_rust.InstEventSemaphore(
                                name=f"splitwait_{n}_{inst.name}",
                                engine=inst.engine,
                                ins=[],
                                outs=[],
                                sync_info=mybir.SyncInfo(on_wait=[w], on_update=[]),
                            )
                        )
                    inst.sync_info = mybir.SyncInfo(
                        on_wait=[waits[-1]], on_update=list(si.on_update)
                    )
                out.append(inst)
            if len(out) != len(insts):
                blk.instructions[:] = out
    return n


def make_work(widths, bases, split_last_base=2):
    """Work items (t, lo, hi, masked, col): masked tails first, bases last,
    the final base split for a short exposed drain."""
    tails = []
    base_pieces = []
    col = 0
    for t in range(TILES):
        W = widths[t]
        bs = bases[t]
        if W > bs:
            tails.append((t, bs, W, True, col))
            col += 1
    last_t = None
    for t in range(TILES):
        if bases[t] > 0:
            last_t = t
    for t in range(TILES):
        bs = bases[t]
        if bs <= 0:
            continue
        if t == last_t and split_last_base > 1 and bs >= 256:
            h = (bs // split_last_base) // 128 * 128
            h = max(h, 128)
            cuts = list(range(0, bs, h))
            for i, lo in enumerate(cuts):
                hi = bs if i == len(cuts) - 1 else min(bs, lo + h)
                if hi > lo:
                    base_pieces.append((t, lo, hi, False, col))
                    col += 1
        else:
            base_pieces.append((t, 0, bs, False, col))
            col += 1
    # interleave masked tails with maskless bases: spreads the compute-heavy
    # pieces across the transfer stream (measured best in the cost model)
    out = []
    for i in range(max(len(tails), len(base_pieces))):
        if i < len(tails):
            out.append(tails[i])
        if i < len(base_pieces):
            out.append(base_pieces[i])
    return out, col


def build_bass(widths, bases, dve_only_last=1, io_bufs=None, m_bufs=None,
               scratch=32768):
    work, ncol = make_work(widths, bases)

    # size pools to fit SBUF for any piece structure
    wp = max((hi - lo) for (_, lo, hi, _, _) in work)
    wm = max(((hi - lo) for (_, lo, hi, mk, _) in work if mk), default=1)
    budget = 148 * 1024 - (MAX_W * 4)
    if m_bufs is None:
        m_bufs = 4 if wm * 4 * 4 <= 40 * 1024 else 2
    if io_bufs is None:
        io_bufs = max(2, min(4, (budget - m_bufs * wm * 4) // (3 * wp * 4)))

    nc = bass.Bass(dynamic_dma_scratch_size=scratch)

    dat = nc.dram_tensor("dat", [ROWS, L], f32, kind="ExternalInput")
    idx = nc.dram_tensor("idx", [P, 3 * TILES], i32, kind="ExternalInput")
    lens = nc.dram_tensor("lens", [P, TILES], f32, kind="ExternalInput")
    res = nc.dram_tensor("res", [P, max(ncol, 1)], f32, kind="ExternalOutput")

    iota_base = min([lo for (_, lo, hi, m, _) in work if m], default=0)

    with tile.TileContext(nc) as tc:
        with (
            tc.tile_pool(name="sc", bufs=1) as sc,
            tc.tile_pool(name="io", bufs=io_bufs) as io,
            tc.tile_pool(name="mp", bufs=m_bufs) as mp,
        ):
            idx_sb = sc.tile([P, 3 * TILES], i32)
            len_sb = sc.tile([P, TILES], f32)
            acc = sc.tile([P, max(ncol, 1)], f32)
            iw = max(MAX_W - iota_base, 1)
            iota_f = sc.tile([P, iw], f32)

            idx_dma = nc.sync.dma_start(out=idx_sb[:], in_=idx[:])
            nc.sync.dma_start(out=len_sb[:], in_=lens[:])

            def emit_gather(t, lo, hi):
                # one single-index gather per array: HW SWDGE reads exactly one
                # offset per partition (multi-index offset tables read as the
                # sim suggests do NOT work on hardware)
                tiles3 = []
                for a, tag in ((0, "x"), (1, "w"), (2, "y")):
                    ti = io.tile([P, hi - lo], f32, tag=tag)
                    nc.gpsimd.indirect_dma_start(
                        out=ti[:], out_offset=None, in_=dat[:],
                        in_offset=bass.IndirectOffsetOnAxis(
                            ap=idx_sb[:, 3 * t + a : 3 * t + a + 1], axis=1
                        ),
                        element_offset=lo,
                    )
                    tiles3.append(ti)
                return tiles3

            def emit_compute(t, g, lo, hi, masked, acc_col, dve_only):
                x = g[0][:]
                w_ = g[1][:]
                y = g[2][:]
                nc.vector.tensor_tensor(
                    out=x, in0=x, in1=y, op=mybir.AluOpType.subtract
                )
                if dve_only:
                    nc.vector.tensor_tensor(
                        out=y, in0=x, in1=x, op=mybir.AluOpType.mult
                    )
                else:
                    nc.scalar.activation(
                        out=y, in_=x, func=mybir.ActivationFunctionType.Square
                    )
                if masked:
                    m = mp.tile([P, hi - lo], f32, tag="m")
                    nc.vector.tensor_scalar(
                        out=m[:],
                        in0=iota_f[:, lo - iota_base : hi - iota_base],
                        scalar1=len_sb[:, t : t + 1],
                        scalar2=None,
                        op0=mybir.AluOpType.is_lt,
                    )
                    nc.vector.tensor_tensor(
                        out=m[:], in0=w_[:], in1=m[:], op=mybir.AluOpType.mult
                    )
                    nc.vector.tensor_tensor(
                        out=w_[:], in0=y[:], in1=m[:], op=mybir.AluOpType.mult
                    )
                else:
                    nc.vector.tensor_tensor(
                        out=w_[:], in0=y[:], in1=w_[:], op=mybir.AluOpType.mult
                    )
                if dve_only:
                    nc.vector.tensor_reduce(
                        out=acc[:, acc_col : acc_col + 1], in_=w_[:],
                        axis=mybir.AxisListType.X, op=mybir.AluOpType.add,
                    )
                else:
                    nc.scalar.activation(
                        out=x, in_=w_[:],
                        func=mybir.ActivationFunctionType.Identity,
                        accum_out=acc[:, acc_col : acc_col + 1],
                    )

            tiles = []
            for i, (t, lo, hi, masked, col) in enumerate(work):
                g = emit_gather(t, lo, hi)
                tiles.append((t, g, lo, hi, masked, col))
                if i == 0:
                    it = nc.gpsimd.iota(
                        iota_f[:], pattern=[[1, iw]], base=iota_base,
                        channel_multiplier=0,
                        allow_small_or_imprecise_dtypes=True,
                    )
                    add_dep_helper(it.ins, idx_dma.ins, reason="iota after idx")
            n = len(tiles)
            for i, item in enumerate(tiles):
                emit_compute(*item, dve_only=(i >= n - dve_only_last))

            nc.sync.dma_start(out=res[:], in_=acc[:])

    legalize_waits(nc)
    return nc, work


def prepare_inputs(fluctuate, ivar, output, overlap_index):
    """Shard + sort samples, build per-core input maps and metadata."""
    flu = np.ascontiguousarray(fluctuate.reshape(B, L), dtype=np.float32)
    ivr = np.ascontiguousarray(ivar.reshape(B, L), dtype=np.float32)
    oup = np.ascontiguousarray(output.reshape(B, L), dtype=np.float32)
    oi = np.asarray(overlap_index)
    s_in = oi[:, 0].astype(np.int64)
    e_in = oi[:, 1].astype(np.int64)
    s_out = oi[:, 2].astype(np.int64)
    all_lens = e_in - s_in

    orders = []
    core_lens = []       # per-core lens in sorted order, [TILES, P]
    for c in range(N_CORES):
        lo = c * BPC
        lens_local = all_lens[lo : lo + BPC]
        # descending: widest tile first, so the exposed drain after the last
        # transfer runs on the narrowest tile
        order = np.argsort(-lens_local, kind="stable")
        orders.append(order)
        core_lens.append(lens_local[order].reshape(TILES, P))

    # shared tile widths (max len, rounded up to 128) and maskless base
    # widths (min len, rounded down to 128) across cores
    widths = []
    bases = []
    for t in range(TILES):
        mx = max(int(core_lens[c][t].max()) for c in range(N_CORES))
        mn = min(int(core_lens[c][t].min()) for c in range(N_CORES))
        w = min(MAX_W, -(-mx // 128) * 128)
        b = max(0, min(mn // 128 * 128, w))
        widths.append(w)
        bases.append(b)

    SEC = (BPC + 1) * L      # element offset between flu/ivr/oup sections
    in_maps = []
    for c in range(N_CORES):
        lo = c * BPC
        order = orders[c]
        rows = order.astype(np.int64)
        g = lo + order
        off_in = rows * L + s_in[g]
        off_out = rows * L + s_out[g]
        idx = np.empty((P, 3 * TILES), dtype=np.int32)
        lens_f = np.empty((P, TILES), dtype=np.float32)
        for t in range(TILES):
            sl = slice(t * P, (t + 1) * P)
            idx[:, 3 * t] = off_in[sl]
            idx[:, 3 * t + 1] = off_in[sl] + SEC
            idx[:, 3 * t + 2] = off_out[sl] + 2 * SEC
            lens_f[:, t] = all_lens[g][sl]

        end = lo + BPC
        pad = np.zeros(L, dtype=np.float32)
        parts = []
        for arr in (flu, ivr, oup):
            if end < B:
                parts.append(arr.reshape(-1)[lo * L : end * L + L])
            else:
                parts.append(
                    np.concatenate([arr.reshape(-1)[lo * L : end * L], pad])
                )
        dat = np.concatenate(parts).reshape(ROWS, L)

        in_maps.append({"dat": dat, "idx": idx, "lens": lens_f})

    return in_maps, widths, bases, core_lens


def finish(results, work, core_lens):
    """Combine per-core per-piece partial sums into the scalar mean."""
    total = 0.0
    for c in range(N_CORES):
        res = results[c]["res"].astype(np.float64)     # [P, ncol]
        sums = np.zeros((TILES, P), dtype=np.float64)
        for (t, lo, hi, masked, col) in work:
            sums[t] += res[:, col]
        lens = core_lens[c].astype(np.float64)
        total += float((sums / lens).sum())
    return np.float32(total / B)


def kernel(fluctuate, ivar, output, overlap_index, _trace=False, **_kw):
    in_maps, widths, bases, core_lens = prepare_inputs(
        fluctuate, ivar, output, overlap_index
    )
    nc, work = build_bass(widths, bases)
    out = run_bass_kernel_spmd(
        nc, in_maps, core_ids=list(range(N_CORES)), trace=_trace
    )
    result = finish(out.results, work, core_lens)
    if _trace:
        return result, out
    return result

